# revision 2
# baseline (speedup 1.0000x reference)
"""LoRA fast-linear Trainium2 kernel.

y = x @ W.T + b + sum_l s_l * (x @ down_l.T) @ up_l.T

Strategy (8 NeuronCores, data-parallel over tokens):
  - Host packs weights:  WT=[IN,OUT] (W transposed),  DT=[IN,L*R] (downs
    flattened+transposed), UT=[L*R,OUT] (scales folded into ups). The LoRA
    rank dims concatenate to exactly 128 = one partition dim.
  - Each core gets 2048 tokens. On-chip, x tiles are transposed feature-major
    via the PE array (fp32), rounded to fp32r, and all matmuls run in fp32r
    (full PE rate, ~13-bit mantissa).  The rank-128 LoRA delta accumulates
    into the same PSUM bank as the base matmul; bias is added by the DVE on
    the PSUM->SBUF copy.
"""

import sys

if "/opt/trn_rl_repo" not in sys.path:
    sys.path.insert(0, "/opt/trn_rl_repo")

import numpy as np

B, S, IN, OUT, L, R = 2, 8192, 2048, 2048, 4, 32
N_CORES = 8
TOKENS = B * S              # 16384
TOK = TOKENS // N_CORES     # 2048 tokens per core
P = 128
KC = IN // P                # 16 contraction chunks
LR = L * R                  # 128 (= P)
ST = 1024                   # tokens per supertile
NST = TOK // ST             # 2
MT = ST // P                # 8 m-tiles (128 tokens) per supertile
NCH = 512                   # out-feature chunk (one fp32 PSUM bank)
NT = OUT // NCH             # 4

_NC_CACHE = {}


def _build_nc():
    import concourse.bacc as bacc
    import concourse.mybir as mybir
    import concourse.tile as tile
    from concourse.masks import make_identity

    dt = mybir.dt
    F32R = dt.float32r

    nc = bacc.Bacc("TRN2", target_bir_lowering=False, debug=False)
    xs = nc.dram_tensor("xs", [TOK, IN], dt.float32, kind="ExternalInput")
    wt = nc.dram_tensor("wt", [IN, OUT], F32R, kind="ExternalInput")
    dts = nc.dram_tensor("dts", [IN, LR], F32R, kind="ExternalInput")
    uts = nc.dram_tensor("uts", [LR, OUT], F32R, kind="ExternalInput")
    bias = nc.dram_tensor("bias", [OUT], dt.float32, kind="ExternalInput")
    ys = nc.dram_tensor("ys", [TOK, OUT], dt.float32, kind="ExternalOutput")

    wt_v = wt.ap().rearrange("(kc p) o -> p kc o", p=P)
    dts_v = dts.ap().rearrange("(kc p) lr -> p kc lr", p=P)

    with tile.TileContext(nc) as tc:
        with (
            tc.tile_pool(name="const", bufs=1) as constp,
            tc.tile_pool(name="wpool", bufs=2) as wpool,
            tc.tile_pool(name="xstage", bufs=2) as xstage,
            tc.tile_pool(name="xtp", bufs=1) as xtp,
            tc.tile_pool(name="ypool", bufs=3) as ypool,
            tc.tile_pool(name="pp_t", bufs=3, space="PSUM") as pp_t,
            tc.tile_pool(name="pp_y", bufs=3, space="PSUM") as pp_y,
            tc.tile_pool(name="pp_l", bufs=2, space="PSUM") as pp_l,
        ):
            ident = constp.tile([P, P], dt.float32)
            make_identity(nc, ident[:])
            dt_sb = constp.tile([P, KC, LR], F32R)
            nc.sync.dma_start(dt_sb[:], dts_v)
            ut_sb = constp.tile([P, OUT], F32R)
            nc.sync.dma_start(ut_sb[:], uts.ap())
            bias_bc = constp.tile([P, OUT], dt.float32)
            nc.sync.dma_start(bias_bc[:], bias.ap()[None, :].to_broadcast((P, OUT)))

            for st in range(NST):
                t0 = st * ST
                xT = xtp.tile([P, KC, ST], F32R, tag="xT")
                tmpT = xtp.tile([P, ST], F32R, tag="tmpT")

                # ---- transpose x supertile to feature-major (PE) ----
                for m in range(MT):
                    stage = xstage.tile([P, IN], dt.float32, tag="stage")
                    nc.sync.dma_start(
                        stage[:], xs.ap()[t0 + m * P : t0 + (m + 1) * P, :]
                    )
                    for kc in range(KC):
                        pst = pp_t.tile([P, P], dt.float32, tag="pst")
                        nc.tensor.transpose(
                            pst[:], stage[:, kc * P : (kc + 1) * P], ident[:]
                        )
                        nc.any.tensor_copy(
                            out=xT[:, kc, m * P : (m + 1) * P], in_=pst[:]
                        )

                # ---- LoRA down-proj: tmpT[lr, t] = DT.T @ xT ----
                for th in range(ST // NCH):
                    pl = pp_l.tile([P, NCH], dt.float32, tag="pl")
                    for kc in range(KC):
                        nc.tensor.matmul(
                            pl[:],
                            dt_sb[:, kc, :],
                            xT[:, kc, th * NCH : (th + 1) * NCH],
                            start=(kc == 0),
                            stop=(kc == KC - 1),
                        )
                    nc.any.tensor_copy(
                        out=tmpT[:, th * NCH : (th + 1) * NCH], in_=pl[:]
                    )

                # ---- base matmul + LoRA up-proj + bias ----
                for n in range(NT):
                    wt_sb = wpool.tile([P, KC, NCH], F32R, tag="wt")
                    nc.sync.dma_start(
                        wt_sb[:], wt_v[:, :, n * NCH : (n + 1) * NCH]
                    )
                    for m in range(MT):
                        py = pp_y.tile([P, NCH], dt.float32, tag="py")
                        for kc in range(KC):
                            nc.tensor.matmul(
                                py[:],
                                xT[:, kc, m * P : (m + 1) * P],
                                wt_sb[:, kc, :],
                                start=(kc == 0),
                                stop=False,
                            )
                        nc.tensor.matmul(
                            py[:],
                            tmpT[:, m * P : (m + 1) * P],
                            ut_sb[:, n * NCH : (n + 1) * NCH],
                            start=False,
                            stop=True,
                        )
                        y_sb = ypool.tile([P, NCH], dt.float32, tag="y")
                        nc.vector.tensor_tensor(
                            y_sb[:],
                            py[:],
                            bias_bc[:, n * NCH : (n + 1) * NCH],
                            mybir.AluOpType.add,
                        )
                        nc.sync.dma_start(
                            ys.ap()[
                                t0 + m * P : t0 + (m + 1) * P,
                                n * NCH : (n + 1) * NCH,
                            ],
                            y_sb[:],
                        )

    nc.compile()
    return nc


def get_nc():
    if "nc" not in _NC_CACHE:
        _NC_CACHE["nc"] = _build_nc()
    return _NC_CACHE["nc"]


def make_in_maps(x, weight, bias, downs, ups, scales):
    x = np.ascontiguousarray(np.asarray(x, dtype=np.float32)).reshape(TOKENS, IN)
    weight = np.asarray(weight, dtype=np.float32)
    bias_np = np.ascontiguousarray(np.asarray(bias, dtype=np.float32))
    downs = np.asarray(downs, dtype=np.float32)
    ups = np.asarray(ups, dtype=np.float32)
    scales = np.asarray(scales, dtype=np.float32)

    wt_np = np.ascontiguousarray(weight.T)                          # [IN, OUT]
    dt_np = np.ascontiguousarray(downs.reshape(LR, IN).T)           # [IN, LR]
    ut_np = np.ascontiguousarray(
        (scales[:, None, None] * ups).transpose(0, 2, 1).reshape(LR, OUT)
    )                                                               # [LR, OUT]

    return [
        {
            "xs": np.ascontiguousarray(x[c * TOK : (c + 1) * TOK]),
            "wt": wt_np,
            "dts": dt_np,
            "uts": ut_np,
            "bias": bias_np,
        }
        for c in range(N_CORES)
    ]


def kernel(x, weight, bias, downs, ups, scales):
    from concourse.bass_utils import run_bass_kernel_spmd

    nc = get_nc()
    in_maps = make_in_maps(x, weight, bias, downs, ups, scales)
    res = run_bass_kernel_spmd(
        nc, in_maps, core_ids=list(range(N_CORES)), trace=False
    )
    y = np.concatenate([res.results[c]["ys"] for c in range(N_CORES)], axis=0)
    return y.reshape(B, S, OUT)


# revision 5
# speedup vs baseline: 3.7756x; 3.7756x over previous
"""LoRA fast-linear Trainium2 kernel.

y = x @ W.T + b + sum_l s_l * (x @ down_l.T) @ up_l.T

Strategy (8 NeuronCores, data-parallel over tokens):
  - Host packs weights:  WT=[IN,OUT] (W transposed),  DT=[IN,L*R] (downs
    flattened+transposed), UT=[L*R,OUT] (scales folded into ups). The LoRA
    rank dims concatenate to exactly 128 = one partition dim.
  - Each core gets 2048 tokens. On-chip, x tiles are transposed feature-major
    via the PE array (fp32), rounded to fp32r, and all matmuls run in fp32r
    (full PE rate, ~13-bit mantissa).  The rank-128 LoRA delta accumulates
    into the same PSUM bank as the base matmul; bias is added by the DVE on
    the PSUM->SBUF copy.
"""

import sys

if "/opt/trn_rl_repo" not in sys.path:
    sys.path.insert(0, "/opt/trn_rl_repo")

import numpy as np

B, S, IN, OUT, L, R = 2, 8192, 2048, 2048, 4, 32
N_CORES = 8
TOKENS = B * S              # 16384
TOK = TOKENS // N_CORES     # 2048 tokens per core
P = 128
KC = IN // P                # 16 contraction chunks
LR = L * R                  # 128 (= P)
ST = 1024                   # tokens per supertile
NST = TOK // ST             # 2
MT = ST // P                # 8 m-tiles (128 tokens) per supertile
NCH = 512                   # out-feature chunk (one fp32 PSUM bank)
NT = OUT // NCH             # 4

_NC_CACHE = {}


def _build_nc(repeat=1):
    """Build the per-core Bass program. ``repeat`` re-runs the whole body
    (same data, same outputs) — used only for device-time measurement via
    timing deltas, since axon has no NTFF profiling."""
    import concourse.bacc as bacc
    import concourse.mybir as mybir
    import concourse.tile as tile
    from concourse.masks import make_identity

    dt = mybir.dt
    F32R = dt.float32r

    nc = bacc.Bacc("TRN2", target_bir_lowering=False, debug=False)
    xs = nc.dram_tensor("xs", [TOK, IN], dt.float32, kind="ExternalInput")
    wt = nc.dram_tensor("wt", [IN, OUT], F32R, kind="ExternalInput")
    dts = nc.dram_tensor("dts", [IN, LR], F32R, kind="ExternalInput")
    uts = nc.dram_tensor("uts", [LR, OUT], F32R, kind="ExternalInput")
    bias = nc.dram_tensor("bias", [OUT], dt.float32, kind="ExternalInput")
    ys = nc.dram_tensor("ys", [TOK, OUT], dt.float32, kind="ExternalOutput")

    wt_v = wt.ap().rearrange("(kc p) o -> p kc o", p=P)
    dts_v = dts.ap().rearrange("(kc p) lr -> p kc lr", p=P)

    with tile.TileContext(nc) as tc:
        with (
            tc.tile_pool(name="const", bufs=1) as constp,
            tc.tile_pool(name="wpool", bufs=2) as wpool,
            tc.tile_pool(name="xstage", bufs=2) as xstage,
            tc.tile_pool(name="xtp", bufs=1) as xtp,
            tc.tile_pool(name="ypool", bufs=3) as ypool,
            tc.tile_pool(name="pp_t", bufs=3, space="PSUM") as pp_t,
            tc.tile_pool(name="pp_y", bufs=3, space="PSUM") as pp_y,
            tc.tile_pool(name="pp_l", bufs=2, space="PSUM") as pp_l,
        ):
            ident = constp.tile([P, P], dt.float32)
            make_identity(nc, ident[:])
            dt_sb = constp.tile([P, KC, LR], F32R)
            nc.sync.dma_start(dt_sb[:], dts_v)
            ut_sb = constp.tile([P, OUT], F32R)
            nc.sync.dma_start(ut_sb[:], uts.ap())
            bias_bc = constp.tile([P, OUT], dt.float32)
            nc.sync.dma_start(bias_bc[:], bias.ap()[None, :].to_broadcast((P, OUT)))

            for st in range(NST * repeat):
                st = st % NST
                t0 = st * ST
                xT = xtp.tile([P, KC, ST], F32R, tag="xT")
                tmpT = xtp.tile([P, ST], F32R, tag="tmpT")

                # ---- transpose x supertile to feature-major (PE) ----
                for m in range(MT):
                    stage = xstage.tile([P, IN], dt.float32, tag="stage")
                    nc.sync.dma_start(
                        stage[:], xs.ap()[t0 + m * P : t0 + (m + 1) * P, :]
                    )
                    for kc in range(KC):
                        pst = pp_t.tile([P, P], dt.float32, tag="pst")
                        nc.tensor.transpose(
                            pst[:], stage[:, kc * P : (kc + 1) * P], ident[:]
                        )
                        nc.any.tensor_copy(
                            out=xT[:, kc, m * P : (m + 1) * P], in_=pst[:]
                        )

                # ---- LoRA down-proj: tmpT[lr, t] = DT.T @ xT ----
                for th in range(ST // NCH):
                    pl = pp_l.tile([P, NCH], dt.float32, tag="pl")
                    for kc in range(KC):
                        nc.tensor.matmul(
                            pl[:],
                            dt_sb[:, kc, :],
                            xT[:, kc, th * NCH : (th + 1) * NCH],
                            start=(kc == 0),
                            stop=(kc == KC - 1),
                        )
                    nc.any.tensor_copy(
                        out=tmpT[:, th * NCH : (th + 1) * NCH], in_=pl[:]
                    )

                # ---- base matmul + LoRA up-proj + bias ----
                for n in range(NT):
                    wt_sb = wpool.tile([P, KC, NCH], F32R, tag="wt")
                    nc.sync.dma_start(
                        wt_sb[:], wt_v[:, :, n * NCH : (n + 1) * NCH]
                    )
                    for m in range(MT):
                        py = pp_y.tile([P, NCH], dt.float32, tag="py")
                        for kc in range(KC):
                            nc.tensor.matmul(
                                py[:],
                                xT[:, kc, m * P : (m + 1) * P],
                                wt_sb[:, kc, :],
                                start=(kc == 0),
                                stop=False,
                            )
                        nc.tensor.matmul(
                            py[:],
                            tmpT[:, m * P : (m + 1) * P],
                            ut_sb[:, n * NCH : (n + 1) * NCH],
                            start=False,
                            stop=True,
                        )
                        y_sb = ypool.tile([P, NCH], dt.float32, tag="y")
                        nc.vector.tensor_tensor(
                            y_sb[:],
                            py[:],
                            bias_bc[:, n * NCH : (n + 1) * NCH],
                            mybir.AluOpType.add,
                        )
                        nc.sync.dma_start(
                            ys.ap()[
                                t0 + m * P : t0 + (m + 1) * P,
                                n * NCH : (n + 1) * NCH,
                            ],
                            y_sb[:],
                        )

    nc.compile()
    return nc


def get_nc(repeat=1):
    key = ("nc", repeat)
    if key not in _NC_CACHE:
        _NC_CACHE[key] = _build_nc(repeat)
    return _NC_CACHE[key]


def make_in_maps(x, weight, bias, downs, ups, scales):
    x = np.ascontiguousarray(np.asarray(x, dtype=np.float32)).reshape(TOKENS, IN)
    weight = np.asarray(weight, dtype=np.float32)
    bias_np = np.ascontiguousarray(np.asarray(bias, dtype=np.float32))
    downs = np.asarray(downs, dtype=np.float32)
    ups = np.asarray(ups, dtype=np.float32)
    scales = np.asarray(scales, dtype=np.float32)

    wt_np = np.ascontiguousarray(weight.T)                          # [IN, OUT]
    dt_np = np.ascontiguousarray(downs.reshape(LR, IN).T)           # [IN, LR]
    ut_np = np.ascontiguousarray(
        (scales[:, None, None] * ups).transpose(0, 2, 1).reshape(LR, OUT)
    )                                                               # [LR, OUT]

    return [
        {
            "xs": np.ascontiguousarray(x[c * TOK : (c + 1) * TOK]),
            "wt": wt_np,
            "dts": dt_np,
            "uts": ut_np,
            "bias": bias_np,
        }
        for c in range(N_CORES)
    ]


def kernel(x, weight, bias, downs, ups, scales):
    from concourse.bass_utils import run_bass_kernel_spmd

    nc = get_nc()
    in_maps = make_in_maps(x, weight, bias, downs, ups, scales)
    res = run_bass_kernel_spmd(
        nc, in_maps, core_ids=list(range(N_CORES)), trace=False
    )
    y = np.concatenate([res.results[c]["ys"] for c in range(N_CORES)], axis=0)
    return y.reshape(B, S, OUT)
